# revision 23
# baseline (speedup 1.0000x reference)
"""BERT self-attention on 8 Trainium2 NeuronCores.

Problem: B=4, S=2048, H=768, nh=12, hd=64.
Sharding: core c -> (batch b = c//2, head-group g = c%2); each core does
1 batch x 6 heads: projections + attention + output slice [2048, 384].

Strategy (v4):
  - Host prep does all layout work (free w.r.t. HW exec time): X is
    permuted so unmasked k-rows come first (k-side shrinks 16 -> 9
    blocks; the q-side is computed in permuted order and un-permuted on
    the host), transposed, cast to bf16, and packed together with the
    pre-transposed weights into one [768, 3200] tensor so the device
    needs only 14 plain DMAs (the HWDGE is a ~630ns/DMA serial server).
  - Intro: the oc0 K/Q projections accumulate i-chunk-by-i-chunk as the
    six row-tiles of the packed tensor land, so the first exp fires
    right after the last K-side DMA instead of after a serial chain.
  - Scores are computed per (kb, q-half): ST[k', q] = K^T.T @ Q^T into
    a ping-pong psum, exp'd on ACT (the bottleneck engine, 1 elem/
    cycle/partition) with the mask folded in as a per-partition bias.
  - PV in natural orientation: out[q, d] += P^T[k', qb].T @ V[k', d|1]
    (65-wide V blocks; col 64 is 1.0 -> softmax denominators). Output
    free size 65 instead of 512 halves PE cost and needs no output
    transposes. PV lags one slot behind its exp.
  - HW psum rule (probe-verified): matmul start=True zeroes the WHOLE
    bank -> only the first group to touch a bank sets start; the rest
    accumulate onto the zeroed remainder with start=False.
  - All other projections (V per-oc, oc1/oc2 K/Q in 256-col halves) are
    stuffed one-per-slot into PE slack inside the ACT-bound phases.
  - Drain per q-half-sweep (DVE copies + reciprocal, Pool scaling), one
    output store per head (2048-descriptor strided DMA, DRAM-side AP
    rearranged so the SBUF side stays partition-major).
"""

import numpy as np
import ml_dtypes

import concourse.bacc as bacc
import concourse.mybir as mybir
from concourse.bass_utils import run_bass_kernel_spmd
from concourse.tile import TileContext

F32 = mybir.dt.float32
BF16 = mybir.dt.bfloat16
BF16_NP = ml_dtypes.bfloat16

S = 2048  # sequence length
H = 768  # hidden
O = 384  # per-core projection width (6 heads * 64)
HD = 64  # head dim
NHEADS = 6  # heads per core
NI = H // 128  # 6 contraction chunks
SB = S // 128  # 16 seq blocks
QC = S // 512  # 4 q chunks
NT_FAST = 9  # k-blocks kept in the compacted build (capacity 1152)

LOG2E = 1.4426950408889634
FXP_C1 = 128.0 * 0.125 * LOG2E
FXP_BIAS = 128.0 * (127.0 - 0.057304959)

# packed input column offsets: [wk | wq | xt | wv]
WK0, WQ0, XT0 = 0, O, 2 * O
WV0 = 2 * O + S
XIN_W = 3 * O + S  # 3200


def build_nc(nt):
    from contextlib import ExitStack

    nc = bacc.Bacc(None, target_bir_lowering=False)
    Exp = mybir.ActivationFunctionType.Exp
    Ident = mybir.ActivationFunctionType.Identity
    KP = nt * 128
    KW = min(KP, S)
    NSLOT = 2 * nt  # slots per head (one exp of [128, 1024] each)

    xin_d = nc.dram_tensor("xin", [H, XIN_W], BF16, kind="ExternalInput")
    aux_d = nc.dram_tensor("aux", [128, 6 + 2 * nt], F32, kind="ExternalInput")
    bv_d = nc.dram_tensor("bvrow", [1, O], BF16, kind="ExternalInput")
    out = nc.dram_tensor("out", [S, O], F32, kind="ExternalOutput")

    # k'-chunk widths for the K projection (multiples of 512 then rest)
    kchunks = []
    off = 0
    while off < KP:
        w = min(512, KP - off)
        kchunks.append((off, w))
        off += w
    NK = len(kchunks)

    with nc.allow_low_precision("bf16 activations by design"), TileContext(nc) as tc:
        with ExitStack() as ctx:
            consts = ctx.enter_context(tc.tile_pool(name="consts", bufs=1))
            data = ctx.enter_context(tc.tile_pool(name="data", bufs=1))
            ptp = ctx.enter_context(tc.tile_pool(name="pt", bufs=8))
            drp = ctx.enter_context(tc.tile_pool(name="dr", bufs=2))
            fxp = ctx.enter_context(tc.tile_pool(name="fxp", bufs=3))
            rcp = ctx.enter_context(tc.tile_pool(name="rc", bufs=4))
            stp = ctx.enter_context(tc.tile_pool(name="st", bufs=2, space="PSUM"))
            pvp = ctx.enter_context(tc.tile_pool(name="pv", bufs=3, space="PSUM"))
            prj = ctx.enter_context(tc.tile_pool(name="prj", bufs=1, space="PSUM"))

            ones_row = consts.tile([1, 128], BF16, tag="ones_row")
            nc.vector.memset(ones_row, 1.0)
            aux = consts.tile([128, 6 + 2 * nt], F32, tag="aux")
            bq2 = aux[:, 0:3]
            bk2 = aux[:, 3:6]
            mask_bias = aux[:, 6 : 6 + nt]
            fxp_c2 = aux[:, 6 + nt : 6 + 2 * nt]
            bvrow = consts.tile([1, O], BF16, tag="bvrow")

            big = [
                data.tile([128, XIN_W], BF16, tag=f"big{i}", name=f"big{i}")
                for i in range(NI)
            ]
            qt = [data.tile([128, S], BF16, tag=f"qt{i}", name=f"qt{i}") for i in range(3)]
            kt = [data.tile([128, KP], BF16, tag=f"kt{i}", name=f"kt{i}") for i in range(3)]
            vt = [
                data.tile([128, NHEADS, HD + 1], BF16, tag=f"v{i}", name=f"v{i}")
                for i in range(nt)
            ]
            osb = data.tile([128, SB, O], F32, tag="osb")

            def wk_s(i, oc):
                return big[i][:, WK0 + oc * 128 : WK0 + (oc + 1) * 128]

            def wq_s(i, oc):
                return big[i][:, WQ0 + oc * 128 : WQ0 + (oc + 1) * 128]

            def xt_s(i, lo, hi):
                return big[i][:, XT0 + lo : XT0 + hi]

            def wv_s(i, lo, hi):
                return big[i][:, WV0 + lo : WV0 + hi]

            # ---- loads: the K+Q-side slab of each row-tile first (gates
            # the intro projections; the HWDGE is a serial ~630ns/DMA
            # server, so the tiny aux loads go behind the big ones), then
            # the x-rest+wv slabs.
            D1 = 2 * O + KW
            for i in range(NI):
                eng = (nc.sync, nc.scalar)[i % 2]
                eng.dma_start(big[i][:, 0:D1], xin_d[i * 128 : (i + 1) * 128, 0:D1])
            nc.sync.dma_start(aux, aux_d[:, :])
            nc.scalar.dma_start(bvrow, bv_d[:, :])
            for i in range(NI):
                eng = (nc.sync, nc.scalar)[i % 2]
                eng.dma_start(
                    big[i][:, D1:XIN_W], xin_d[i * 128 : (i + 1) * 128, D1:XIN_W]
                )



            # ---- intro: oc0 K/Q projections, i-chunk interleaved so each
            # psum group accumulates as its row-tile arrives. Each group
            # owns a full psum bank.
            intro_views = []
            tA = stp.tile([128, 1024], F32, tag="st", name="introA")
            intro_views += [tA[:, 0:512], tA[:, 512:1024]]
            tB = stp.tile([128, 1024], F32, tag="st", name="introB")
            intro_views += [tB[:, 0:512], tB[:, 512:1024]]
            tP = prj.tile([128, 512], F32, tag="prj", name="introP")
            intro_views.append(tP)

            pass1 = [("k", ci) for ci in range(NK)]
            pass2 = []
            for qc in range(QC):
                (pass1 if (qc + 1) * 512 <= KW else pass2).append(("q", qc))
            for j in range(len(pass1) + len(pass2) - 5):
                intro_views.append(pvp.tile([128, 512], F32, tag="pv", name="introV"))

            def intro_mm(view, g, i):
                kind, idx = g
                if kind == "k":
                    coff, cw = kchunks[idx]
                    lhsT, rhs, w = wk_s(i, 0), xt_s(i, coff, coff + cw), cw
                else:
                    lhsT, rhs, w = (
                        wq_s(i, 0),
                        xt_s(i, idx * 512, (idx + 1) * 512),
                        512,
                    )
                nc.tensor.matmul(
                    view[:, 0:w], lhsT, rhs, start=(i == 0), stop=(i == NI - 1)
                )

            for i in range(NI):
                for gi, g in enumerate(pass1):
                    intro_mm(intro_views[gi], g, i)
            # copies+bias: alternate ACT/DVE, K chunk0 and Q qc0/qc1 first
            order = sorted(range(len(pass1)), key=lambda gi: (pass1[gi][1], pass1[gi][0]))
            for j, gi in enumerate(order):
                kind, idx = pass1[gi]
                view = intro_views[gi]
                if kind == "k":
                    coff, cw = kchunks[idx]
                    dst, b = kt[0][:, coff : coff + cw], bk2
                else:
                    cw = 512
                    dst, b = qt[0][:, idx * 512 : (idx + 1) * 512], bq2
                if j % 2 == 0:
                    nc.scalar.activation(dst, view[:, 0:cw], Ident, bias=b[:, 0:1])
                else:
                    nc.vector.tensor_scalar_add(dst, view[:, 0:cw], b[:, 0:1])
            for gi, g in enumerate(pass2):
                view = intro_views[len(pass1) + gi]
                for i in range(NI):
                    intro_mm(view, g, i)
                _, qc = g
                nc.vector.tensor_scalar_add(
                    qt[0][:, qc * 512 : (qc + 1) * 512], view, bq2[:, 0:1]
                )

            # ---- stuffed work units (emitted one per slot in PE slack) ---
            pending_prj = {}

            def vproj_unit(kb, oc):
                # V for 2 heads (one oc chunk) of k-block kb; bias via a
                # ones-row matmul; col 64 of each head block set to 1.0.
                ps = prj.tile([128, 512], F32, tag="prj", name="psv")
                for i in range(NI):
                    nc.tensor.matmul(
                        ps[:, 0:128],
                        xt_s(i, kb * 128, (kb + 1) * 128),
                        wv_s(i, oc * 128, (oc + 1) * 128),
                        start=(i == 0),
                        stop=False,
                    )
                nc.tensor.matmul(
                    ps[:, 0:128],
                    ones_row,
                    bvrow[:, oc * 128 : (oc + 1) * 128],
                    start=False,
                    stop=True,
                )
                nc.vector.tensor_copy(
                    vt[kb][:, 2 * oc : 2 * oc + 2, 0:HD],
                    ps[:, 0:128].rearrange("p (h d) -> p h d", d=HD),
                )
                nc.gpsimd.memset(vt[kb][:, 2 * oc : 2 * oc + 2, HD : HD + 1], 1.0)

            def kq_unit(kind, oc, idx, half, last):
                # one 256-col half (or a short K tail) of a K/Q projection
                # group; the group's first matmul start=True wipes the bank.
                key = (kind, oc, idx)
                first = key not in pending_prj
                if first:
                    pending_prj[key] = prj.tile([128, 512], F32, tag="prj", name="psg")
                ps = pending_prj[key]
                if kind == "k":
                    coff, cw = kchunks[idx]
                else:
                    coff, cw = idx * 512, 512
                off = 0 if half in (None, 0) else 256
                w = cw if half is None else 256
                for i in range(NI):
                    if kind == "k":
                        rhs = xt_s(i, coff + off, coff + off + w)
                        lhsT = wk_s(i, oc)
                    else:
                        rhs = xt_s(i, coff + off, coff + off + w)
                        lhsT = wq_s(i, oc)
                    nc.tensor.matmul(
                        ps[:, off : off + w],
                        lhsT,
                        rhs,
                        start=(first and i == 0),
                        stop=(last and i == NI - 1),
                        skip_group_check=True,
                    )
                if last:
                    del pending_prj[key]
                    if kind == "k":
                        nc.vector.tensor_scalar_add(
                            kt[oc][:, coff : coff + cw], ps[:, 0:cw],
                            bk2[:, oc : oc + 1],
                        )
                    else:
                        nc.vector.tensor_scalar_add(
                            qt[oc][:, coff : coff + cw], ps[:, 0:cw],
                            bq2[:, oc : oc + 1],
                        )

            def kq_units(oc):
                u = []
                for ci, (coff, cw) in enumerate(kchunks):
                    if cw > 256:
                        u.append(("k", oc, ci, 0, False))
                        u.append(("k", oc, ci, 1, True))
                    else:
                        u.append(("k", oc, ci, None, True))
                for qc in range(QC):
                    u.append(("q", oc, qc, 0, False))
                    u.append(("q", oc, qc, 1, True))
                return u

            # slot schedule: vproj for head-pair p rides head 2p's first
            # sweep (slot = kb, one slot ahead of the PV that consumes it);
            # oc1/oc2 K/Q halves spread every other slot in the windows
            # after their vproj sweeps, finishing before their head pair.
            stuff = {}
            for p in range(3):
                for kb in range(nt):
                    stuff[(2 * p) * NSLOT + kb] = ("v", kb, p)
            for oc, w0 in ((1, nt), (2, 5 * nt)):
                for j, u in enumerate(kq_units(oc)):
                    stuff[w0 + 2 * j] = ("kq",) + u

            def emit_stuffed(u):
                if u[0] == "v":
                    vproj_unit(u[1], u[2])
                else:
                    kq_unit(*u[1:])

            # ---- attention --------------------------------------------
            pvg_sets = {}

            def emit_pv(ph, pqh, pkb, ppt):
                if pqh == 0 and pkb == 0:
                    pvg_sets[ph] = [
                        pvp.tile([128, 512], F32, tag="pv", name="pvg")
                        for _ in range(3)
                    ]
                pvg = pvg_sets[ph]
                for j in range(8):
                    qb = pqh * 8 + j
                    nc.tensor.matmul(
                        pvg[qb // 7][:, (qb % 7) * 65 : (qb % 7) * 65 + 65],
                        ppt[:, j * 128 : (j + 1) * 128],
                        vt[pkb][:, ph, :],
                        start=(pkb == 0 and qb in (0, 7, 14)),
                        stop=(pkb == nt - 1),
                        skip_group_check=True,
                    )
                if pkb == nt - 1:
                    drain(ph, pqh)

            dr_cur = {}

            def drain(ph, pqh):
                # copy the finished psum regions out fast (frees banks for
                # the next head), reciprocal of the denominator column,
                # scale on Pool, one strided store per head.
                pvg = pvg_sets[ph]
                if pqh == 0:
                    dr_cur[ph] = drp.tile([128, 1040], F32, tag="dr", name="dr")
                dr = dr_cur[ph]
                if pqh == 0:
                    nc.vector.tensor_copy(dr[:, 0:455], pvg[0][:, 0:455])
                    nc.vector.tensor_copy(dr[:, 455:520], pvg[1][:, 0:65])
                else:
                    nc.vector.tensor_copy(dr[:, 520:910], pvg[1][:, 65:455])
                    nc.vector.tensor_copy(dr[:, 910:1040], pvg[2][:, 0:130])
                drv = dr.rearrange("p (b c) -> p b c", c=65)
                rc = rcp.tile([128, 8], F32, tag="rc", name="rc")
                nc.vector.reciprocal(rc, drv[:, pqh * 8 : (pqh + 1) * 8, 64])
                for j in range(8):
                    qb = pqh * 8 + j
                    eng = nc.gpsimd if j % 2 == 0 else nc.vector
                    eng.tensor_scalar_mul(
                        osb[:, qb, ph * HD : (ph + 1) * HD],
                        dr[:, qb * 65 : qb * 65 + 64],
                        rc[:, j : j + 1],
                    )
                if pqh == 1:
                    del pvg_sets[ph]
                    del dr_cur[ph]
                for qb0 in ((0,) if pqh == 0 else (8,)):
                    nc.sync.dma_start(
                        out[qb0 * 128 : (qb0 + 8) * 128, ph * HD : (ph + 1) * HD]
                        .rearrange("(b p) c -> p b c", p=128),
                        osb[:, qb0 : qb0 + 8, ph * HD : (ph + 1) * HD],
                    )

            prev = None
            slot = 0
            for h in range(NHEADS):
                oc, base = h // 2, (h % 2) * 64
                qt_h = qt[oc][base : base + 64, :]
                kt_h = kt[oc][base : base + 64, :]
                for qh in range(2):
                    for kb in range(nt):
                        pt = ptp.tile([128, 1024], BF16, tag="pt", name="pt")
                        st = stp.tile([128, 1024], F32, tag="st", name="st")
                        for qq in range(2):
                            qcc = qh * 2 + qq
                            nc.tensor.matmul(
                                st[:, qq * 512 : (qq + 1) * 512],
                                kt_h[:, kb * 128 : (kb + 1) * 128],
                                qt_h[:, qcc * 512 : (qcc + 1) * 512],
                                start=True,
                                stop=True,
                            )
                        if False:
                            # Schraudolph fast-exp off the ACT engine: bf16
                            # bits of exp(s/8+b) ~= s*C1 + c2[p], written as
                            # uint16 (negative -> clamp 0 kills masked rows).
                            # op1 reads PSUM, so it stays on DVE; the convert
                            # goes to Pool for the kb==8 tile.
                            tmp = fxp.tile([128, 1024], F32, tag="tmp", name="tmp")
                            nc.vector.tensor_scalar(
                                out=tmp, in0=st, scalar1=FXP_C1,
                                scalar2=fxp_c2[:, kb : kb + 1],
                                op0=mybir.AluOpType.mult, op1=mybir.AluOpType.add,
                            )
                            eng = nc.vector if kb == 4 else nc.gpsimd
                            eng.tensor_copy(pt.bitcast(mybir.dt.uint16), tmp)
                        else:
                            nc.scalar.activation(
                                pt, st, Exp, bias=mask_bias[:, kb : kb + 1],
                                scale=0.125,
                            )
                        if slot in stuff:
                            emit_stuffed(stuff[slot])
                        if prev is not None:
                            emit_pv(*prev)
                        prev = (h, qh, kb, pt)
                        slot += 1
            emit_pv(*prev)

    nc.finalize()
    return nc


_NC_CACHE = {}


def _get_nc(nt):
    if nt not in _NC_CACHE:
        _NC_CACHE[nt] = build_nc(nt)
    return _NC_CACHE[nt]


def _pick_nt(inputs):
    am = np.asarray(inputs["attention_mask"], dtype=np.float32)
    max_keep = int((am[:, 0, 0, :] >= 0).sum(axis=1).max())
    return NT_FAST if max_keep <= NT_FAST * 128 else SB


def _prep(inputs, nt):
    hs = np.asarray(inputs["hidden_states"], dtype=np.float32)
    am = np.asarray(inputs["attention_mask"], dtype=np.float32)
    Ws = {k: np.asarray(inputs[k], dtype=np.float32) for k in ("Wq", "Wk", "Wv")}
    bs = {k: np.asarray(inputs[k], dtype=np.float32) for k in ("bq", "bk", "bv")}

    in_maps, perms = [], []
    for c in range(8):
        b, g = c // 2, c % 2
        sl = slice(g * O, (g + 1) * O)
        m = am[b, 0, 0, :]
        if nt != SB:
            keep = np.nonzero(m >= 0)[0]
            drop = np.nonzero(m < 0)[0]
            perm = np.concatenate([keep, drop])
        else:
            perm = np.arange(S)
        perms.append(perm)
        mp = m[perm[: nt * 128]]
        mask_bias = np.where(mp < 0, np.float32(-10000.0), np.float32(0.0))
        xin = np.concatenate(
            [
                Ws["Wk"][sl].T.astype(BF16_NP),
                Ws["Wq"][sl].T.astype(BF16_NP),
                hs[b][perm].T.astype(BF16_NP),
                Ws["Wv"][sl].T.astype(BF16_NP),
            ],
            axis=1,
        )
        c2 = (128.0 * 1.4426950408889634 * mask_bias
              + np.float32(128.0 * (127.0 - 0.057304959)))
        auxm = np.concatenate(
            [
                bs["bq"][sl].reshape(3, 128).T,
                bs["bk"][sl].reshape(3, 128).T,
                mask_bias.reshape(nt, 128).T.astype(np.float32),
                c2.reshape(nt, 128).T.astype(np.float32),
            ],
            axis=1,
        )
        in_maps.append(
            {
                "xin": np.ascontiguousarray(xin),
                "aux": np.ascontiguousarray(auxm),
                "bvrow": np.ascontiguousarray(bs["bv"][sl].astype(BF16_NP)[None, :]),
            }
        )
    return in_maps, perms


def kernel(**inputs):
    nt = _pick_nt(inputs)
    nc = _get_nc(nt)
    in_maps, perms = _prep(inputs, nt)
    res = run_bass_kernel_spmd(nc, in_maps, core_ids=list(range(8)))
    outp = np.empty((4, S, H), dtype=np.float32)
    for c in range(8):
        b, g = c // 2, c % 2
        outp[b][perms[c], g * O : (g + 1) * O] = res.results[c]["out"]
    return outp


# revision 29
# speedup vs baseline: 1.0056x; 1.0056x over previous
"""BERT self-attention on 8 Trainium2 NeuronCores.

Problem: B=4, S=2048, H=768, nh=12, hd=64.
Sharding: core c -> (batch b = c//2, head-group g = c%2); each core does
1 batch x 6 heads: projections + attention + output slice [2048, 384].

Strategy (v4):
  - Host prep does all layout work (free w.r.t. HW exec time): X is
    permuted so unmasked k-rows come first (k-side shrinks 16 -> 9
    blocks; the q-side is computed in permuted order and un-permuted on
    the host), transposed, cast to bf16, and packed together with the
    pre-transposed weights into one [768, 3200] tensor so the device
    needs only 14 plain DMAs (the HWDGE is a ~630ns/DMA serial server).
  - Intro: the oc0 K/Q projections accumulate i-chunk-by-i-chunk as the
    six row-tiles of the packed tensor land, so the first exp fires
    right after the last K-side DMA instead of after a serial chain.
  - Scores are computed per (kb, q-half): ST[k', q] = K^T.T @ Q^T into
    a ping-pong psum, exp'd on ACT (the bottleneck engine, 1 elem/
    cycle/partition) with the mask folded in as a per-partition bias.
  - PV in natural orientation: out[q, d] += P^T[k', qb].T @ V[k', d|1]
    (65-wide V blocks; col 64 is 1.0 -> softmax denominators). Output
    free size 65 instead of 512 halves PE cost and needs no output
    transposes. PV lags one slot behind its exp.
  - HW psum rule (probe-verified): matmul start=True zeroes the WHOLE
    bank -> only the first group to touch a bank sets start; the rest
    accumulate onto the zeroed remainder with start=False.
  - All other projections (V per-oc, oc1/oc2 K/Q in 256-col halves) are
    stuffed one-per-slot into PE slack inside the ACT-bound phases.
  - Drain per q-half-sweep (DVE copies + reciprocal, Pool scaling), one
    output store per head (2048-descriptor strided DMA, DRAM-side AP
    rearranged so the SBUF side stays partition-major).
"""

import numpy as np
import ml_dtypes

import concourse.bacc as bacc
import concourse.mybir as mybir
from concourse.bass_utils import run_bass_kernel_spmd
from concourse.tile import TileContext

F32 = mybir.dt.float32
BF16 = mybir.dt.bfloat16
BF16_NP = ml_dtypes.bfloat16

S = 2048  # sequence length
H = 768  # hidden
O = 384  # per-core projection width (6 heads * 64)
HD = 64  # head dim
NHEADS = 6  # heads per core
NI = H // 128  # 6 contraction chunks
SB = S // 128  # 16 seq blocks
QC = S // 512  # 4 q chunks
NT_FAST = 9  # k-blocks kept in the compacted build (capacity 1152)

LOG2E = 1.4426950408889634
FXP_C1 = 128.0 * 0.125 * LOG2E
FXP_BIAS = 128.0 * (127.0 - 0.057304959)

# packed input column offsets: [wk | wq | xt | wv]
WK0, WQ0, XT0 = 0, O, 2 * O
WV0 = 2 * O + S
XIN_W = 3 * O + S  # 3200


def build_nc(nt):
    from contextlib import ExitStack

    nc = bacc.Bacc(None, target_bir_lowering=False)
    Exp = mybir.ActivationFunctionType.Exp
    Ident = mybir.ActivationFunctionType.Identity
    KP = nt * 128
    KW = min(KP, S)
    NSLOT = 2 * nt  # slots per head (one exp of [128, 1024] each)

    xin_d = nc.dram_tensor("xin", [H, XIN_W], BF16, kind="ExternalInput")
    aux_d = nc.dram_tensor("aux", [128, 6 + 2 * nt], F32, kind="ExternalInput")
    bv_d = nc.dram_tensor("bvrow", [1, O], BF16, kind="ExternalInput")
    out = nc.dram_tensor("out", [S, O], F32, kind="ExternalOutput")

    # k'-chunk widths for the K projection (multiples of 512 then rest)
    kchunks = []
    off = 0
    while off < KP:
        w = min(512, KP - off)
        kchunks.append((off, w))
        off += w
    NK = len(kchunks)

    with nc.allow_low_precision("bf16 activations by design"), TileContext(nc) as tc:
        with ExitStack() as ctx:
            consts = ctx.enter_context(tc.tile_pool(name="consts", bufs=1))
            data = ctx.enter_context(tc.tile_pool(name="data", bufs=1))
            ptp = ctx.enter_context(tc.tile_pool(name="pt", bufs=12))
            drp = ctx.enter_context(tc.tile_pool(name="dr", bufs=4))
            fxp = ctx.enter_context(tc.tile_pool(name="fxp", bufs=3))
            rcp = ctx.enter_context(tc.tile_pool(name="rc", bufs=6))
            stp = ctx.enter_context(tc.tile_pool(name="st", bufs=2, space="PSUM"))
            pvp = ctx.enter_context(tc.tile_pool(name="pv", bufs=3, space="PSUM"))
            prj = ctx.enter_context(tc.tile_pool(name="prj", bufs=1, space="PSUM"))

            ones_row = consts.tile([1, 128], BF16, tag="ones_row")
            nc.vector.memset(ones_row, 1.0)
            aux = consts.tile([128, 6 + 2 * nt], F32, tag="aux")
            bq2 = aux[:, 0:3]
            bk2 = aux[:, 3:6]
            mask_bias = aux[:, 6 : 6 + nt]
            fxp_c2 = aux[:, 6 + nt : 6 + 2 * nt]
            bvrow = consts.tile([1, O], BF16, tag="bvrow")

            big = [
                data.tile([128, XIN_W], BF16, tag=f"big{i}", name=f"big{i}")
                for i in range(NI)
            ]
            qt = [data.tile([128, S], BF16, tag=f"qt{i}", name=f"qt{i}") for i in range(3)]
            kt = [data.tile([128, KP], BF16, tag=f"kt{i}", name=f"kt{i}") for i in range(3)]
            vt = [
                data.tile([128, NHEADS, HD + 1], BF16, tag=f"v{i}", name=f"v{i}")
                for i in range(nt)
            ]
            osb = data.tile([128, SB, O], F32, tag="osb")

            def wk_s(i, oc):
                return big[i][:, WK0 + oc * 128 : WK0 + (oc + 1) * 128]

            def wq_s(i, oc):
                return big[i][:, WQ0 + oc * 128 : WQ0 + (oc + 1) * 128]

            def xt_s(i, lo, hi):
                return big[i][:, XT0 + lo : XT0 + hi]

            def wv_s(i, lo, hi):
                return big[i][:, WV0 + lo : WV0 + hi]

            # ---- loads: the K+Q-side slab of each row-tile first (gates
            # the intro projections; the HWDGE is a serial ~630ns/DMA
            # server, so the tiny aux loads go behind the big ones), then
            # the x-rest+wv slabs.
            D1 = 2 * O + KW
            for i in range(NI):
                eng = (nc.sync, nc.scalar)[i % 2]
                eng.dma_start(big[i][:, 0:D1], xin_d[i * 128 : (i + 1) * 128, 0:D1])
            nc.sync.dma_start(aux, aux_d[:, :])
            nc.scalar.dma_start(bvrow, bv_d[:, :])
            for i in range(NI):
                eng = (nc.sync, nc.scalar)[i % 2]
                eng.dma_start(
                    big[i][:, D1:XIN_W], xin_d[i * 128 : (i + 1) * 128, D1:XIN_W]
                )



            # ---- intro: oc0 K/Q projections, i-chunk interleaved so each
            # psum group accumulates as its row-tile arrives. Each group
            # owns a full psum bank.
            intro_views = []
            tA = stp.tile([128, 1024], F32, tag="st", name="introA")
            intro_views += [tA[:, 0:512], tA[:, 512:1024]]
            tB = stp.tile([128, 1024], F32, tag="st", name="introB")
            intro_views += [tB[:, 0:512], tB[:, 512:1024]]
            tP = prj.tile([128, 512], F32, tag="prj", name="introP")
            intro_views.append(tP)

            pass1 = [("k", ci) for ci in range(NK)]
            pass2 = []
            for qc in range(QC):
                (pass1 if (qc + 1) * 512 <= KW else pass2).append(("q", qc))
            for j in range(len(pass1) + len(pass2) - 5):
                intro_views.append(pvp.tile([128, 512], F32, tag="pv", name="introV"))

            def intro_mm(view, g, i):
                kind, idx = g
                if kind == "k":
                    coff, cw = kchunks[idx]
                    lhsT, rhs, w = wk_s(i, 0), xt_s(i, coff, coff + cw), cw
                else:
                    lhsT, rhs, w = (
                        wq_s(i, 0),
                        xt_s(i, idx * 512, (idx + 1) * 512),
                        512,
                    )
                nc.tensor.matmul(
                    view[:, 0:w], lhsT, rhs, start=(i == 0), stop=(i == NI - 1)
                )

            for i in range(NI):
                for gi, g in enumerate(pass1):
                    intro_mm(intro_views[gi], g, i)
            # copies+bias: alternate ACT/DVE, K chunk0 and Q qc0/qc1 first
            order = sorted(range(len(pass1)), key=lambda gi: (pass1[gi][1], pass1[gi][0]))
            for j, gi in enumerate(order):
                kind, idx = pass1[gi]
                view = intro_views[gi]
                if kind == "k":
                    coff, cw = kchunks[idx]
                    dst, b = kt[0][:, coff : coff + cw], bk2
                else:
                    cw = 512
                    dst, b = qt[0][:, idx * 512 : (idx + 1) * 512], bq2
                if j % 2 == 0:
                    nc.scalar.activation(dst, view[:, 0:cw], Ident, bias=b[:, 0:1])
                else:
                    nc.vector.tensor_scalar_add(dst, view[:, 0:cw], b[:, 0:1])
            for gi, g in enumerate(pass2):
                view = intro_views[len(pass1) + gi]
                for i in range(NI):
                    intro_mm(view, g, i)
                _, qc = g
                nc.vector.tensor_scalar_add(
                    qt[0][:, qc * 512 : (qc + 1) * 512], view, bq2[:, 0:1]
                )

            # ---- stuffed work units (emitted one per slot in PE slack) ---
            pending_prj = {}

            def vproj_unit(kb, oc):
                # V for 2 heads (one oc chunk) of k-block kb; bias via a
                # ones-row matmul; col 64 of each head block set to 1.0.
                ps = prj.tile([128, 512], F32, tag="prj", name="psv")
                for i in range(NI):
                    nc.tensor.matmul(
                        ps[:, 0:128],
                        xt_s(i, kb * 128, (kb + 1) * 128),
                        wv_s(i, oc * 128, (oc + 1) * 128),
                        start=(i == 0),
                        stop=False,
                    )
                nc.tensor.matmul(
                    ps[:, 0:128],
                    ones_row,
                    bvrow[:, oc * 128 : (oc + 1) * 128],
                    start=False,
                    stop=True,
                )
                nc.vector.tensor_copy(
                    vt[kb][:, 2 * oc : 2 * oc + 2, 0:HD],
                    ps[:, 0:128].rearrange("p (h d) -> p h d", d=HD),
                )
                nc.gpsimd.memset(vt[kb][:, 2 * oc : 2 * oc + 2, HD : HD + 1], 1.0)

            def kq_unit(kind, oc, idx, half, last):
                # one 256-col half (or a short K tail) of a K/Q projection
                # group; the group's first matmul start=True wipes the bank.
                key = (kind, oc, idx)
                first = key not in pending_prj
                if first:
                    pending_prj[key] = prj.tile([128, 512], F32, tag="prj", name="psg")
                ps = pending_prj[key]
                if kind == "k":
                    coff, cw = kchunks[idx]
                else:
                    coff, cw = idx * 512, 512
                off = 0 if half in (None, 0) else 256
                w = cw if half is None else 256
                for i in range(NI):
                    if kind == "k":
                        rhs = xt_s(i, coff + off, coff + off + w)
                        lhsT = wk_s(i, oc)
                    else:
                        rhs = xt_s(i, coff + off, coff + off + w)
                        lhsT = wq_s(i, oc)
                    nc.tensor.matmul(
                        ps[:, off : off + w],
                        lhsT,
                        rhs,
                        start=(first and i == 0),
                        stop=(last and i == NI - 1),
                        skip_group_check=True,
                    )
                if last:
                    del pending_prj[key]
                    if kind == "k":
                        nc.vector.tensor_scalar_add(
                            kt[oc][:, coff : coff + cw], ps[:, 0:cw],
                            bk2[:, oc : oc + 1],
                        )
                    else:
                        nc.vector.tensor_scalar_add(
                            qt[oc][:, coff : coff + cw], ps[:, 0:cw],
                            bq2[:, oc : oc + 1],
                        )

            def kq_units(oc):
                u = []
                for ci, (coff, cw) in enumerate(kchunks):
                    if cw > 256:
                        u.append(("k", oc, ci, 0, False))
                        u.append(("k", oc, ci, 1, True))
                    else:
                        u.append(("k", oc, ci, None, True))
                for qc in range(QC):
                    u.append(("q", oc, qc, 0, False))
                    u.append(("q", oc, qc, 1, True))
                return u

            # slot schedule: vproj for head-pair p rides head 2p's first
            # sweep (slot = kb, one slot ahead of the PV that consumes it);
            # oc1/oc2 K/Q halves spread every other slot in the windows
            # after their vproj sweeps, finishing before their head pair.
            stuff = {}
            for p in range(3):
                for kb in range(nt):
                    stuff[(2 * p) * NSLOT + kb] = ("v", kb, p)
            for oc, w0 in ((1, nt), (2, 5 * nt)):
                for j, u in enumerate(kq_units(oc)):
                    stuff[w0 + 2 * j] = ("kq",) + u

            def emit_stuffed(u):
                if u[0] == "v":
                    vproj_unit(u[1], u[2])
                else:
                    kq_unit(*u[1:])

            # ---- attention --------------------------------------------
            pvg_sets = {}

            def emit_pv(ph, pqh, pkb, ppt):
                if pqh == 0 and pkb == 0:
                    pvg_sets[ph] = [
                        pvp.tile([128, 512], F32, tag="pv", name="pvg")
                        for _ in range(3)
                    ]
                pvg = pvg_sets[ph]
                for j in range(8):
                    qb = pqh * 8 + j
                    nc.tensor.matmul(
                        pvg[qb // 7][:, (qb % 7) * 65 : (qb % 7) * 65 + 65],
                        ppt[:, j * 128 : (j + 1) * 128],
                        vt[pkb][:, ph, :],
                        start=(pkb == 0 and qb in (0, 7, 14)),
                        stop=(pkb == nt - 1),
                        skip_group_check=True,
                    )
                if pkb == nt - 1:
                    drain(ph, pqh)

            dr_cur = {}

            def drain(ph, pqh):
                # copy the finished psum regions out fast (frees banks for
                # the next head), reciprocal of the denominator column,
                # scale on Pool, one strided store per head.
                pvg = pvg_sets[ph]
                if pqh == 0:
                    dr_cur[ph] = drp.tile([128, 1040], F32, tag="dr", name="dr")
                dr = dr_cur[ph]
                if pqh == 0:
                    nc.vector.tensor_copy(dr[:, 0:455], pvg[0][:, 0:455])
                    nc.vector.tensor_copy(dr[:, 455:520], pvg[1][:, 0:65])
                else:
                    nc.vector.tensor_copy(dr[:, 520:910], pvg[1][:, 65:455])
                    nc.vector.tensor_copy(dr[:, 910:1040], pvg[2][:, 0:130])
                drv = dr.rearrange("p (b c) -> p b c", c=65)
                rc = rcp.tile([128, 8], F32, tag="rc", name="rc")
                nc.vector.reciprocal(rc, drv[:, pqh * 8 : (pqh + 1) * 8, 64])
                for j in range(8):
                    qb = pqh * 8 + j
                    eng = nc.gpsimd if j % 2 == 0 else nc.vector
                    eng.tensor_scalar_mul(
                        osb[:, qb, ph * HD : (ph + 1) * HD],
                        dr[:, qb * 65 : qb * 65 + 64],
                        rc[:, j : j + 1],
                    )
                if pqh == 1:
                    del pvg_sets[ph]
                    del dr_cur[ph]
                for qb0 in ((0, 4) if pqh == 0 else (8, 12)):
                    nc.sync.dma_start(
                        out[qb0 * 128 : (qb0 + 4) * 128, ph * HD : (ph + 1) * HD]
                        .rearrange("(b p) c -> p b c", p=128),
                        osb[:, qb0 : qb0 + 4, ph * HD : (ph + 1) * HD],
                    )

            prev = None
            slot = 0
            for h in range(NHEADS):
                oc, base = h // 2, (h % 2) * 64
                qt_h = qt[oc][base : base + 64, :]
                kt_h = kt[oc][base : base + 64, :]
                for qh in range(2):
                    for kb in range(nt):
                        pt = ptp.tile([128, 1024], BF16, tag="pt", name="pt")
                        st = stp.tile([128, 1024], F32, tag="st", name="st")
                        for qq in range(2):
                            qcc = qh * 2 + qq
                            nc.tensor.matmul(
                                st[:, qq * 512 : (qq + 1) * 512],
                                kt_h[:, kb * 128 : (kb + 1) * 128],
                                qt_h[:, qcc * 512 : (qcc + 1) * 512],
                                start=True,
                                stop=True,
                            )
                        if False:
                            # Schraudolph fast-exp off the ACT engine: bf16
                            # bits of exp(s/8+b) ~= s*C1 + c2[p], written as
                            # uint16 (negative -> clamp 0 kills masked rows).
                            # op1 reads PSUM, so it stays on DVE; the convert
                            # goes to Pool for the kb==8 tile.
                            tmp = fxp.tile([128, 1024], F32, tag="tmp", name="tmp")
                            nc.vector.tensor_scalar(
                                out=tmp, in0=st, scalar1=FXP_C1,
                                scalar2=fxp_c2[:, kb : kb + 1],
                                op0=mybir.AluOpType.mult, op1=mybir.AluOpType.add,
                            )
                            eng = nc.vector if kb == 4 else nc.gpsimd
                            eng.tensor_copy(pt.bitcast(mybir.dt.uint16), tmp)
                        else:
                            nc.scalar.activation(
                                pt, st, Exp, bias=mask_bias[:, kb : kb + 1],
                                scale=0.125,
                            )
                        if slot in stuff:
                            emit_stuffed(stuff[slot])
                        if prev is not None:
                            emit_pv(*prev)
                        prev = (h, qh, kb, pt)
                        slot += 1
            emit_pv(*prev)

    nc.finalize()
    return nc


_NC_CACHE = {}


def _get_nc(nt):
    if nt not in _NC_CACHE:
        _NC_CACHE[nt] = build_nc(nt)
    return _NC_CACHE[nt]


def _pick_nt(inputs):
    am = np.asarray(inputs["attention_mask"], dtype=np.float32)
    max_keep = int((am[:, 0, 0, :] >= 0).sum(axis=1).max())
    return NT_FAST if max_keep <= NT_FAST * 128 else SB


def _prep(inputs, nt):
    hs = np.asarray(inputs["hidden_states"], dtype=np.float32)
    am = np.asarray(inputs["attention_mask"], dtype=np.float32)
    Ws = {k: np.asarray(inputs[k], dtype=np.float32) for k in ("Wq", "Wk", "Wv")}
    bs = {k: np.asarray(inputs[k], dtype=np.float32) for k in ("bq", "bk", "bv")}

    in_maps, perms = [], []
    for c in range(8):
        b, g = c // 2, c % 2
        sl = slice(g * O, (g + 1) * O)
        m = am[b, 0, 0, :]
        if nt != SB:
            keep = np.nonzero(m >= 0)[0]
            drop = np.nonzero(m < 0)[0]
            perm = np.concatenate([keep, drop])
        else:
            perm = np.arange(S)
        perms.append(perm)
        mp = m[perm[: nt * 128]]
        mask_bias = np.where(mp < 0, np.float32(-10000.0), np.float32(0.0))
        xin = np.concatenate(
            [
                Ws["Wk"][sl].T.astype(BF16_NP),
                Ws["Wq"][sl].T.astype(BF16_NP),
                hs[b][perm].T.astype(BF16_NP),
                Ws["Wv"][sl].T.astype(BF16_NP),
            ],
            axis=1,
        )
        c2 = (128.0 * 1.4426950408889634 * mask_bias
              + np.float32(128.0 * (127.0 - 0.057304959)))
        auxm = np.concatenate(
            [
                bs["bq"][sl].reshape(3, 128).T,
                bs["bk"][sl].reshape(3, 128).T,
                mask_bias.reshape(nt, 128).T.astype(np.float32),
                c2.reshape(nt, 128).T.astype(np.float32),
            ],
            axis=1,
        )
        in_maps.append(
            {
                "xin": np.ascontiguousarray(xin),
                "aux": np.ascontiguousarray(auxm),
                "bvrow": np.ascontiguousarray(bs["bv"][sl].astype(BF16_NP)[None, :]),
            }
        )
    return in_maps, perms


def kernel(**inputs):
    nt = _pick_nt(inputs)
    nc = _get_nc(nt)
    in_maps, perms = _prep(inputs, nt)
    res = run_bass_kernel_spmd(nc, in_maps, core_ids=list(range(8)))
    outp = np.empty((4, S, H), dtype=np.float32)
    for c in range(8):
        b, g = c // 2, c % 2
        outp[b][perms[c], g * O : (g + 1) * O] = res.results[c]["out"]
    return outp


# revision 33
# speedup vs baseline: 1.0080x; 1.0024x over previous
"""BERT self-attention on 8 Trainium2 NeuronCores.

Problem: B=4, S=2048, H=768, nh=12, hd=64.
Sharding: core c -> (batch b = c//2, head-group g = c%2); each core does
1 batch x 6 heads: projections + attention + output slice [2048, 384].

Strategy (v4):
  - Host prep does all layout work (free w.r.t. HW exec time): X is
    permuted so unmasked k-rows come first (k-side shrinks 16 -> 9
    blocks; the q-side is computed in permuted order and un-permuted on
    the host), transposed, cast to bf16, and packed together with the
    pre-transposed weights into one [768, 3200] tensor so the device
    needs only 14 plain DMAs (the HWDGE is a ~630ns/DMA serial server).
  - Intro: the oc0 K/Q projections accumulate i-chunk-by-i-chunk as the
    six row-tiles of the packed tensor land, so the first exp fires
    right after the last K-side DMA instead of after a serial chain.
  - Scores are computed per (kb, q-half): ST[k', q] = K^T.T @ Q^T into
    a ping-pong psum, exp'd on ACT (the bottleneck engine, 1 elem/
    cycle/partition) with the mask folded in as a per-partition bias.
  - PV in natural orientation: out[q, d] += P^T[k', qb].T @ V[k', d|1]
    (65-wide V blocks; col 64 is 1.0 -> softmax denominators). Output
    free size 65 instead of 512 halves PE cost and needs no output
    transposes. PV lags one slot behind its exp.
  - HW psum rule (probe-verified): matmul start=True zeroes the WHOLE
    bank -> only the first group to touch a bank sets start; the rest
    accumulate onto the zeroed remainder with start=False.
  - All other projections (V per-oc, oc1/oc2 K/Q in 256-col halves) are
    stuffed one-per-slot into PE slack inside the ACT-bound phases.
  - Drain per q-half-sweep (DVE copies + reciprocal, Pool scaling), one
    output store per head (2048-descriptor strided DMA, DRAM-side AP
    rearranged so the SBUF side stays partition-major).
"""

import numpy as np
import ml_dtypes

import concourse.bacc as bacc
import concourse.mybir as mybir
from concourse.bass_utils import run_bass_kernel_spmd
from concourse.tile import TileContext

F32 = mybir.dt.float32
BF16 = mybir.dt.bfloat16
BF16_NP = ml_dtypes.bfloat16

S = 2048  # sequence length
H = 768  # hidden
O = 384  # per-core projection width (6 heads * 64)
HD = 64  # head dim
NHEADS = 6  # heads per core
NI = H // 128  # 6 contraction chunks
SB = S // 128  # 16 seq blocks
QC = S // 512  # 4 q chunks
NT_FAST = 9  # k-blocks kept in the compacted build (capacity 1152)

LOG2E = 1.4426950408889634
FXP_C1 = 128.0 * 0.125 * LOG2E
FXP_BIAS = 128.0 * (127.0 - 0.057304959)

# packed input column offsets: [wk | wq | xt | wv]
WK0, WQ0, XT0 = 0, O, 2 * O
WV0 = 2 * O + S
XIN_W = 3 * O + S  # 3200


def build_nc(nt):
    from contextlib import ExitStack

    nc = bacc.Bacc(None, target_bir_lowering=False)
    Exp = mybir.ActivationFunctionType.Exp
    Ident = mybir.ActivationFunctionType.Identity
    KP = nt * 128
    KW = min(KP, S)
    NSLOT = 2 * nt  # slots per head (one exp of [128, 1024] each)

    xin_d = nc.dram_tensor("xin", [H, XIN_W], BF16, kind="ExternalInput")
    aux_d = nc.dram_tensor("aux", [128, 6 + 2 * nt], F32, kind="ExternalInput")
    bv_d = nc.dram_tensor("bvrow", [1, O], BF16, kind="ExternalInput")
    out = nc.dram_tensor("out", [S, O], F32, kind="ExternalOutput")

    # k'-chunk widths for the K projection (multiples of 512 then rest)
    kchunks = []
    off = 0
    while off < KP:
        w = min(512, KP - off)
        kchunks.append((off, w))
        off += w
    NK = len(kchunks)

    with nc.allow_low_precision("bf16 activations by design"), TileContext(nc) as tc:
        with ExitStack() as ctx:
            consts = ctx.enter_context(tc.tile_pool(name="consts", bufs=1))
            data = ctx.enter_context(tc.tile_pool(name="data", bufs=1))
            ptp = ctx.enter_context(tc.tile_pool(name="pt", bufs=12))
            drp = ctx.enter_context(tc.tile_pool(name="dr", bufs=4))
            fxp = ctx.enter_context(tc.tile_pool(name="fxp", bufs=3))
            rcp = ctx.enter_context(tc.tile_pool(name="rc", bufs=6))
            stp = ctx.enter_context(tc.tile_pool(name="st", bufs=2, space="PSUM"))
            pvp = ctx.enter_context(tc.tile_pool(name="pv", bufs=3, space="PSUM"))
            prj = ctx.enter_context(tc.tile_pool(name="prj", bufs=1, space="PSUM"))

            ones_row = consts.tile([1, 128], BF16, tag="ones_row")
            nc.vector.memset(ones_row, 1.0)
            aux = consts.tile([128, 6 + 2 * nt], F32, tag="aux")
            bq2 = aux[:, 0:3]
            bk2 = aux[:, 3:6]
            mask_bias = aux[:, 6 : 6 + nt]
            fxp_c2 = aux[:, 6 + nt : 6 + 2 * nt]
            bvrow = consts.tile([1, O], BF16, tag="bvrow")

            big = [
                data.tile([128, XIN_W], BF16, tag=f"big{i}", name=f"big{i}")
                for i in range(NI)
            ]
            qt = [data.tile([128, S], BF16, tag=f"qt{i}", name=f"qt{i}") for i in range(3)]
            kt = [data.tile([128, KP], BF16, tag=f"kt{i}", name=f"kt{i}") for i in range(3)]
            vt = [
                data.tile([128, NHEADS, HD + 1], BF16, tag=f"v{i}", name=f"v{i}")
                for i in range(nt)
            ]
            osb = data.tile([128, SB, O], F32, tag="osb")

            def wk_s(i, oc):
                return big[i][:, WK0 + oc * 128 : WK0 + (oc + 1) * 128]

            def wq_s(i, oc):
                return big[i][:, WQ0 + oc * 128 : WQ0 + (oc + 1) * 128]

            def xt_s(i, lo, hi):
                return big[i][:, XT0 + lo : XT0 + hi]

            def wv_s(i, lo, hi):
                return big[i][:, WV0 + lo : WV0 + hi]

            # ---- loads: the K+Q-side slab of each row-tile first (gates
            # the intro projections; the HWDGE is a serial ~630ns/DMA
            # server, so the tiny aux loads go behind the big ones), then
            # the x-rest+wv slabs.
            D1 = 2 * O + KW
            for i in range(NI):
                eng = (nc.sync, nc.scalar)[i % 2]
                eng.dma_start(big[i][:, 0:D1], xin_d[i * 128 : (i + 1) * 128, 0:D1])
            nc.sync.dma_start(aux, aux_d[:, :])
            nc.scalar.dma_start(bvrow, bv_d[:, :])
            for i in range(NI):
                eng = (nc.sync, nc.scalar)[i % 2]
                eng.dma_start(
                    big[i][:, D1:XIN_W], xin_d[i * 128 : (i + 1) * 128, D1:XIN_W]
                )



            # ---- intro: oc0 K/Q projections, i-chunk interleaved so each
            # psum group accumulates as its row-tile arrives. Each group
            # owns a full psum bank.
            intro_views = []
            tA = stp.tile([128, 1024], F32, tag="st", name="introA")
            intro_views += [tA[:, 0:512], tA[:, 512:1024]]
            tB = stp.tile([128, 1024], F32, tag="st", name="introB")
            intro_views += [tB[:, 0:512], tB[:, 512:1024]]
            tP = prj.tile([128, 512], F32, tag="prj", name="introP")
            intro_views.append(tP)

            pass1 = [("k", ci) for ci in range(NK)]
            pass2 = []
            for qc in range(QC):
                (pass1 if (qc + 1) * 512 <= KW else pass2).append(("q", qc))
            for j in range(len(pass1) + len(pass2) - 5):
                intro_views.append(pvp.tile([128, 512], F32, tag="pv", name="introV"))

            def intro_mm(view, g, i):
                kind, idx = g
                if kind == "k":
                    coff, cw = kchunks[idx]
                    lhsT, rhs, w = wk_s(i, 0), xt_s(i, coff, coff + cw), cw
                else:
                    lhsT, rhs, w = (
                        wq_s(i, 0),
                        xt_s(i, idx * 512, (idx + 1) * 512),
                        512,
                    )
                nc.tensor.matmul(
                    view[:, 0:w], lhsT, rhs, start=(i == 0), stop=(i == NI - 1)
                )

            for i in range(NI):
                for gi, g in enumerate(pass1):
                    intro_mm(intro_views[gi], g, i)
            # copies+bias: alternate ACT/DVE, K chunk0 and Q qc0/qc1 first
            order = sorted(range(len(pass1)), key=lambda gi: (pass1[gi][1], pass1[gi][0]))
            for j, gi in enumerate(order):
                kind, idx = pass1[gi]
                view = intro_views[gi]
                if kind == "k":
                    coff, cw = kchunks[idx]
                    dst, b = kt[0][:, coff : coff + cw], bk2
                else:
                    cw = 512
                    dst, b = qt[0][:, idx * 512 : (idx + 1) * 512], bq2
                if j % 2 == 0:
                    nc.scalar.activation(dst, view[:, 0:cw], Ident, bias=b[:, 0:1])
                else:
                    nc.vector.tensor_scalar_add(dst, view[:, 0:cw], b[:, 0:1])
            for gi, g in enumerate(pass2):
                view = intro_views[len(pass1) + gi]
                for i in range(NI):
                    intro_mm(view, g, i)
                _, qc = g
                nc.vector.tensor_scalar_add(
                    qt[0][:, qc * 512 : (qc + 1) * 512], view, bq2[:, 0:1]
                )

            # ---- stuffed work units (emitted one per slot in PE slack) ---
            pending_prj = {}

            def vproj_unit(kb, oc):
                # V for 2 heads (one oc chunk) of k-block kb; bias via a
                # ones-row matmul; col 64 of each head block set to 1.0.
                ps = prj.tile([128, 512], F32, tag="prj", name="psv")
                for i in range(NI):
                    nc.tensor.matmul(
                        ps[:, 0:128],
                        xt_s(i, kb * 128, (kb + 1) * 128),
                        wv_s(i, oc * 128, (oc + 1) * 128),
                        start=(i == 0),
                        stop=False,
                    )
                nc.tensor.matmul(
                    ps[:, 0:128],
                    ones_row,
                    bvrow[:, oc * 128 : (oc + 1) * 128],
                    start=False,
                    stop=True,
                )
                nc.vector.tensor_copy(
                    vt[kb][:, 2 * oc : 2 * oc + 2, 0:HD],
                    ps[:, 0:128].rearrange("p (h d) -> p h d", d=HD),
                )
                nc.gpsimd.memset(vt[kb][:, 2 * oc : 2 * oc + 2, HD : HD + 1], 1.0)

            def kq_unit(kind, oc, idx, half, last):
                # one 256-col half (or a short K tail) of a K/Q projection
                # group; the group's first matmul start=True wipes the bank.
                key = (kind, oc, idx)
                first = key not in pending_prj
                if first:
                    pending_prj[key] = prj.tile([128, 512], F32, tag="prj", name="psg")
                ps = pending_prj[key]
                if kind == "k":
                    coff, cw = kchunks[idx]
                else:
                    coff, cw = idx * 512, 512
                off = 0 if half in (None, 0) else 256
                w = cw if half is None else 256
                for i in range(NI):
                    if kind == "k":
                        rhs = xt_s(i, coff + off, coff + off + w)
                        lhsT = wk_s(i, oc)
                    else:
                        rhs = xt_s(i, coff + off, coff + off + w)
                        lhsT = wq_s(i, oc)
                    nc.tensor.matmul(
                        ps[:, off : off + w],
                        lhsT,
                        rhs,
                        start=(first and i == 0),
                        stop=(last and i == NI - 1),
                        skip_group_check=True,
                    )
                if last:
                    del pending_prj[key]
                    if kind == "k":
                        nc.vector.tensor_scalar_add(
                            kt[oc][:, coff : coff + cw], ps[:, 0:cw],
                            bk2[:, oc : oc + 1],
                        )
                    else:
                        nc.vector.tensor_scalar_add(
                            qt[oc][:, coff : coff + cw], ps[:, 0:cw],
                            bq2[:, oc : oc + 1],
                        )

            def kq_units(oc):
                u = []
                for ci, (coff, cw) in enumerate(kchunks):
                    if cw > 256:
                        u.append(("k", oc, ci, 0, False))
                        u.append(("k", oc, ci, 1, True))
                    else:
                        u.append(("k", oc, ci, None, True))
                for qc in range(QC):
                    u.append(("q", oc, qc, 0, False))
                    u.append(("q", oc, qc, 1, True))
                return u

            # slot schedule: vproj for head-pair p rides head 2p's first
            # sweep (slot = kb, one slot ahead of the PV that consumes it);
            # oc1/oc2 K/Q halves spread every other slot in the windows
            # after their vproj sweeps, finishing before their head pair.
            stuff = {}
            for p in range(3):
                for kb in range(nt):
                    stuff[(2 * p) * NSLOT + kb] = ("v", kb, p)
            for oc, w0 in ((1, nt), (2, 5 * nt)):
                for j, u in enumerate(kq_units(oc)):
                    stuff[w0 + 2 * j] = ("kq",) + u

            def emit_stuffed(u):
                if u[0] == "v":
                    vproj_unit(u[1], u[2])
                else:
                    kq_unit(*u[1:])

            # ---- attention --------------------------------------------
            pvg_sets = {}

            def emit_pv(ph, pqh, pkb, ppt):
                if pqh == 0 and pkb == 0:
                    pvg_sets[ph] = [
                        pvp.tile([128, 512], F32, tag="pv", name="pvg")
                        for _ in range(3)
                    ]
                pvg = pvg_sets[ph]
                for j in range(8):
                    qb = pqh * 8 + j
                    nc.tensor.matmul(
                        pvg[qb // 7][:, (qb % 7) * 65 : (qb % 7) * 65 + 65],
                        ppt[:, j * 128 : (j + 1) * 128],
                        vt[pkb][:, ph, :],
                        start=(pkb == 0 and qb in (0, 7, 14)),
                        stop=(pkb == nt - 1),
                        skip_group_check=True,
                    )
                if pkb == nt - 1:
                    drain(ph, pqh)

            dr_cur = {}

            def drain(ph, pqh):
                # copy the finished psum regions out fast (frees banks for
                # the next head), reciprocal of the denominator column,
                # scale on Pool, one strided store per head.
                pvg = pvg_sets[ph]
                if pqh == 0:
                    dr_cur[ph] = drp.tile([128, 1040], F32, tag="dr", name="dr")
                dr = dr_cur[ph]
                drv = dr.rearrange("p (b c) -> p b c", c=65)
                last = ph == NHEADS - 1 and pqh == 1
                if pqh == 0:
                    chunks = [
                        ([(0, 455, pvg[0][:, 0:455]), (455, 520, pvg[1][:, 0:65])],
                         0)
                    ]
                elif last:
                    # final head: drain in 4-qb chunks so the first store
                    # fires while the second chunk is still scaling.
                    chunks = [
                        ([(520, 780, pvg[1][:, 65:325])], 8),
                        ([(780, 910, pvg[1][:, 325:455]),
                          (910, 1040, pvg[2][:, 0:130])], 12),
                    ]
                else:
                    chunks = [
                        ([(520, 910, pvg[1][:, 65:455]),
                          (910, 1040, pvg[2][:, 0:130])], 8)
                    ]
                for copies, qbase in chunks:
                    for lo, hi, srcv in copies:
                        nc.vector.tensor_copy(dr[:, lo:hi], srcv)
                    n = 4 if last else 8
                    rc = rcp.tile([128, 8], F32, tag="rc", name="rc")
                    nc.vector.reciprocal(
                        rc[:, 0:n], drv[:, qbase : qbase + n, 64]
                    )
                    for j in range(n):
                        qb = qbase + j
                        eng = nc.gpsimd if j % 2 == 0 else nc.vector
                        eng.tensor_scalar_mul(
                            osb[:, qb, ph * HD : (ph + 1) * HD],
                            dr[:, qb * 65 : qb * 65 + 64],
                            rc[:, j : j + 1],
                        )
                    if last:
                        nc.sync.dma_start(
                            out[qbase * 128 : (qbase + 4) * 128,
                                ph * HD : (ph + 1) * HD]
                            .rearrange("(b p) c -> p b c", p=128),
                            osb[:, qbase : qbase + 4, ph * HD : (ph + 1) * HD],
                        )
                if pqh == 1:
                    del pvg_sets[ph]
                    del dr_cur[ph]
                if not last:
                    for qb0 in ((0, 4) if pqh == 0 else (8, 12)):
                        nc.sync.dma_start(
                            out[qb0 * 128 : (qb0 + 4) * 128, ph * HD : (ph + 1) * HD]
                            .rearrange("(b p) c -> p b c", p=128),
                            osb[:, qb0 : qb0 + 4, ph * HD : (ph + 1) * HD],
                        )

            prev = None
            slot = 0
            for h in range(NHEADS):
                oc, base = h // 2, (h % 2) * 64
                qt_h = qt[oc][base : base + 64, :]
                kt_h = kt[oc][base : base + 64, :]
                for qh in range(2):
                    for kb in range(nt):
                        pt = ptp.tile([128, 1024], BF16, tag="pt", name="pt")
                        st = stp.tile([128, 1024], F32, tag="st", name="st")
                        for qq in range(2):
                            qcc = qh * 2 + qq
                            nc.tensor.matmul(
                                st[:, qq * 512 : (qq + 1) * 512],
                                kt_h[:, kb * 128 : (kb + 1) * 128],
                                qt_h[:, qcc * 512 : (qcc + 1) * 512],
                                start=True,
                                stop=True,
                            )
                        if False:
                            # Schraudolph fast-exp off the ACT engine: bf16
                            # bits of exp(s/8+b) ~= s*C1 + c2[p], written as
                            # uint16 (negative -> clamp 0 kills masked rows).
                            # op1 reads PSUM, so it stays on DVE; the convert
                            # goes to Pool for the kb==8 tile.
                            tmp = fxp.tile([128, 1024], F32, tag="tmp", name="tmp")
                            nc.vector.tensor_scalar(
                                out=tmp, in0=st, scalar1=FXP_C1,
                                scalar2=fxp_c2[:, kb : kb + 1],
                                op0=mybir.AluOpType.mult, op1=mybir.AluOpType.add,
                            )
                            eng = nc.vector if kb == 4 else nc.gpsimd
                            eng.tensor_copy(pt.bitcast(mybir.dt.uint16), tmp)
                        else:
                            nc.scalar.activation(
                                pt, st, Exp, bias=mask_bias[:, kb : kb + 1],
                                scale=0.125,
                            )
                        if slot in stuff:
                            emit_stuffed(stuff[slot])
                        if prev is not None:
                            emit_pv(*prev)
                        prev = (h, qh, kb, pt)
                        slot += 1
            emit_pv(*prev)

    nc.finalize()
    return nc


_NC_CACHE = {}


def _get_nc(nt):
    if nt not in _NC_CACHE:
        _NC_CACHE[nt] = build_nc(nt)
    return _NC_CACHE[nt]


def _pick_nt(inputs):
    am = np.asarray(inputs["attention_mask"], dtype=np.float32)
    max_keep = int((am[:, 0, 0, :] >= 0).sum(axis=1).max())
    return NT_FAST if max_keep <= NT_FAST * 128 else SB


def _prep(inputs, nt):
    hs = np.asarray(inputs["hidden_states"], dtype=np.float32)
    am = np.asarray(inputs["attention_mask"], dtype=np.float32)
    Ws = {k: np.asarray(inputs[k], dtype=np.float32) for k in ("Wq", "Wk", "Wv")}
    bs = {k: np.asarray(inputs[k], dtype=np.float32) for k in ("bq", "bk", "bv")}

    in_maps, perms = [], []
    for c in range(8):
        b, g = c // 2, c % 2
        sl = slice(g * O, (g + 1) * O)
        m = am[b, 0, 0, :]
        if nt != SB:
            keep = np.nonzero(m >= 0)[0]
            drop = np.nonzero(m < 0)[0]
            perm = np.concatenate([keep, drop])
        else:
            perm = np.arange(S)
        perms.append(perm)
        mp = m[perm[: nt * 128]]
        mask_bias = np.where(mp < 0, np.float32(-10000.0), np.float32(0.0))
        xin = np.concatenate(
            [
                Ws["Wk"][sl].T.astype(BF16_NP),
                Ws["Wq"][sl].T.astype(BF16_NP),
                hs[b][perm].T.astype(BF16_NP),
                Ws["Wv"][sl].T.astype(BF16_NP),
            ],
            axis=1,
        )
        c2 = (128.0 * 1.4426950408889634 * mask_bias
              + np.float32(128.0 * (127.0 - 0.057304959)))
        auxm = np.concatenate(
            [
                bs["bq"][sl].reshape(3, 128).T,
                bs["bk"][sl].reshape(3, 128).T,
                mask_bias.reshape(nt, 128).T.astype(np.float32),
                c2.reshape(nt, 128).T.astype(np.float32),
            ],
            axis=1,
        )
        in_maps.append(
            {
                "xin": np.ascontiguousarray(xin),
                "aux": np.ascontiguousarray(auxm),
                "bvrow": np.ascontiguousarray(bs["bv"][sl].astype(BF16_NP)[None, :]),
            }
        )
    return in_maps, perms


def kernel(**inputs):
    nt = _pick_nt(inputs)
    nc = _get_nc(nt)
    in_maps, perms = _prep(inputs, nt)
    res = run_bass_kernel_spmd(nc, in_maps, core_ids=list(range(8)))
    outp = np.empty((4, S, H), dtype=np.float32)
    for c in range(8):
        b, g = c // 2, c % 2
        outp[b][perms[c], g * O : (g + 1) * O] = res.results[c]["out"]
    return outp
